# revision 10
# baseline (speedup 1.0000x reference)
"""DeepChebNet (3-layer ChebConv K=3 + MLP head) on 8 Trainium2 NeuronCores.

v2 strategy (1D node partition, AllGather-overlapped two-phase propagate):
  - 50000 nodes padded to 51200, two 25600-row half-tables (A/B); each core
    owns 3200 nodes of each half (50 x 128-node blocks: 25 "lo" + 25 "hi").
  - Edges grouped by (dst block, src half); idx streams use trailing -1
    padding so the SWDGE ucode skips pad rows (no wasted gather packets).
  - Each propagate runs in two phases: phase A accumulates all blocks'
    src-half-A edge tiles (PSUM) and evicts; phase B accumulates half-B
    tiles and add-evicts.  Phase A only needs the AllGather of table half
    A, phase B only half B -> each AllGather hides behind the other
    half's gather+matmul work.
  - cheb_out (W0/W1/W2 matmuls + bias/ReLU), the PE transpose to node-major
    rows, the table stores, and the final MLP head are fused into the
    phase-B per-block loop (software-pipelined lag-1/lag-2) so the PE never
    idles long enough to re-throttle (HAM) and collectives fire mid-loop.
  - smat loads ride the sync HWDGE ring; table/y stores ride the scalar
    HWDGE ring (avoids FIFO head-of-line blocking between the two).
"""
import numpy as np

import concourse.bacc as bacc
import concourse.bass as bass
import concourse.mybir as mybir
import concourse.tile as tile
from concourse.bass_utils import run_bass_kernel_spmd
from concourse.masks import make_identity

# problem constants (hardcoded per harness contract)
N_NODES = 50000
N_EDGES = 800000
D = 128
K = 3
BN_EPS = 1e-5

N_CORES = 8
P = 128
N_PAD = 51200
HALF = 25600            # rows per half-table (< 32768: int16-indexable)
HSLAB = 3200            # per-core nodes per half
BLK_NODES = 6400        # per-core nodes
N_BLOCKS = 50           # per-core 128-node blocks (25 lo + 25 hi)
N_HB = 25               # blocks per half per core
N_QUEUES = 4
TRIM_IDX = False  # -1 idx trimming crashes the gather ucode on this stack
BPC = 4              # dst blocks per gather call (amortizes SWDGE call overhead)
N_CALLS = (N_BLOCKS + BPC - 1) // BPC
N_CACHE_BLKS = 8     # smat slabs for blocks 0..7 (both halves) stay SBUF-resident; call-aligned

F16 = mybir.dt.float16
F32 = mybir.dt.float32
npf16 = np.float16


def _owner_block(n):
    """global node id -> (core, block 0..49) under the lo/hi layout."""
    lo = n < HALF
    core = np.where(lo, n // HSLAB, (n - HALF) // HSLAB)
    blk = np.where(lo, (n % HSLAB) // P, N_HB + ((n - HALF) % HSLAB) // P)
    return core, blk


def _preprocess(edge_index, edge_weight):
    """Graph partition + per-core edge streams (dst-block / src-half)."""
    src = np.asarray(edge_index[0], dtype=np.int64)
    dst = np.asarray(edge_index[1], dtype=np.int64)
    w = np.asarray(edge_weight, dtype=np.float32)

    deg = np.bincount(src, weights=w.astype(np.float64), minlength=N_NODES)
    deg = deg.astype(np.float32)
    degs = np.sqrt(np.maximum(deg, 1e-38))
    dinv = np.where(deg > 0, 1.0 / degs, 0.0).astype(np.float32)
    norm = (-dinv[src] * w * dinv[dst]).astype(np.float32)

    core, blk = _owner_block(dst)
    half = (src >= HALF).astype(np.int64)
    key = (core * N_BLOCKS + blk) * 2 + half
    order = np.argsort(key, kind="stable")
    src_s, dst_s, norm_s, key_s = src[order], dst[order], norm[order], key[order]

    n_groups = N_CORES * N_BLOCKS * 2    # 800 (core, block, half) groups
    bounds = np.searchsorted(key_s, np.arange(n_groups + 1))
    counts = bounds[1:] - bounds[:-1]
    t_half = max(1, int(np.max((counts + P - 1) // P)))  # tiles per group

    idx_all, smat_all = [], []
    n_tiles = N_BLOCKS * 2 * t_half
    for c in range(N_CORES):
        gslots = t_half * P
        n_slots = N_BLOCKS * 2 * gslots
        e_src = np.zeros(n_slots, dtype=np.int16)
        e_dstl = np.zeros(n_slots, dtype=np.int64)
        e_norm = np.zeros(n_slots, dtype=np.float32)
        e_live = np.zeros(n_slots, dtype=bool)
        for b in range(N_BLOCKS):
            for h in range(2):
                gidx = (c * N_BLOCKS + b) * 2 + h
                lo, hi = bounds[gidx], bounds[gidx + 1]
                n = hi - lo
                base = (b * 2 + h) * gslots
                e_src[base:base + n] = (src_s[lo:hi] - h * HALF).astype(np.int16)
                e_dstl[base:base + n] = dst_s[lo:hi] % P
                e_norm[base:base + n] = norm_s[lo:hi]
                e_live[base:base + n] = True
                # pad to a full 128-row tile with real row-0 gathers; -1 is
                # only legal for whole trailing empty tiles (ucode trims
                # those; mid-tile -1 hangs).  Exception: the first 8 gather
                # calls (phase A of the first propagate, blocks 0-7, half 0)
                # pad fully with row 0 so every slot of all 8 gather buffers
                # is written once -> later trimmed slots only ever see stale
                # *finite* data (their S column is 0, so stale*0=0, but
                # NaN*0 would poison PSUM).
                keep = max(((n + P - 1) // P) * P, P)
                if TRIM_IDX and not (h == 0 and b < 8):
                    e_src[base + keep:base + gslots] = -1
        # int16 idx stream: per (h, b) group, flat i -> (row i%16, col i//16),
        # replicated across the 8 groups of 16 partitions.  Groups are laid
        # out (half, block)-contiguous so one gather call spans BPC blocks.
        n_grp = N_BLOCKS * 2
        per_grp = np.transpose(
            e_src.reshape(N_BLOCKS, 2, gslots // 16, 16), (1, 0, 3, 2)
        ).reshape(n_grp, 16, gslots // 16)
        arr = np.concatenate([per_grp[i] for i in range(n_grp)], axis=1)
        idx16 = np.zeros((P, n_grp * (gslots // 16)), dtype=np.int16)
        for gs in range(8):
            idx16[gs * 16:(gs + 1) * 16, :] = arr
        idx_all.append(np.ascontiguousarray(idx16))
        # precomputed selection matrices: smat[p, gt*P + j] =
        #   norm_e if (tile gt, lane p) holds edge e with dst_local j
        slot = np.nonzero(e_live)[0]
        gt, lane = slot // P, slot % P
        s_all = np.zeros(n_tiles * P * P, dtype=np.float16)
        s_all[(gt * P + lane) * P + e_dstl[slot]] = e_norm[slot]
        # reorder tiles (b, h, t) -> (h, b, t) to match the idx layout
        smat = np.ascontiguousarray(
            s_all.reshape(N_BLOCKS, 2, t_half, P, P).transpose(3, 1, 0, 2, 4)
            .reshape(P, -1))
        smat_all.append(smat)
    return t_half, idx_all, smat_all


def _build_program(t_half, b2_val):
    """Build the SPMD Bass program (identical across cores)."""
    nc = bacc.Bacc("TRN2", target_bir_lowering=False, debug=False,
                   num_devices=N_CORES, num_swdge_queues=N_QUEUES)

    tp = t_half                  # tiles per (block, half) group
    gw = tp * P                  # gather / S width per group
    gcols = gw // 16             # idx columns per group
    n_tiles = N_BLOCKS * 2 * tp

    # ---- I/O -----------------------------------------------------------
    xA = nc.dram_tensor("xA", [HALF, D], F16, kind="ExternalInput")
    xB = nc.dram_tensor("xB", [HALF, D], F16, kind="ExternalInput")
    x0fm = nc.dram_tensor("x0fm", [P, BLK_NODES], F16, kind="ExternalInput")
    idx_d = nc.dram_tensor("idx", [P, 2 * N_BLOCKS * gcols], mybir.dt.int16,
                           kind="ExternalInput")
    smat_d = nc.dram_tensor("smat", [P, n_tiles * P], F16,
                            kind="ExternalInput")
    wts_d = nc.dram_tensor("wts", [P, 9 * D + D + 1], F16, kind="ExternalInput")
    bias_d = nc.dram_tensor("bias", [P, 4], F32, kind="ExternalInput")
    y_d = nc.dram_tensor("y", [1, BLK_NODES], F32, kind="ExternalOutput")

    tabsA = [nc.dram_tensor(f"tabA{i}", [HALF, D], F16, addr_space="Shared")
             for i in range(5)]
    tabsB = [nc.dram_tensor(f"tabB{i}", [HALF, D], F16, addr_space="Shared")
             for i in range(5)]
    rg = [list(range(N_CORES))]

    with tile.TileContext(nc) as tc:
        with (
            tc.tile_pool(name="const", bufs=1) as constp,
            tc.tile_pool(name="big", bufs=1) as bigp,
            tc.tile_pool(name="gat", bufs=3) as gatp,
            tc.tile_pool(name="sel", bufs=2) as selp,
            tc.tile_pool(name="nm", bufs=4) as nmp,
            tc.tile_pool(name="ps", bufs=4, space="PSUM") as psp,
            tc.tile_pool(name="pc", bufs=2, space="PSUM") as pcp,
            tc.tile_pool(name="pt", bufs=1, space="PSUM") as pstp,
            tc.tile_pool(name="p2", bufs=1, space="PSUM") as p2p,
            tc.tile_pool(name="dram", bufs=1, space="DRAM") as dramp,
        ):
            # ---- load constants -----------------------------------------
            idx_t = constp.tile([P, 2 * N_BLOCKS * gcols], mybir.dt.int16)
            wts_t = constp.tile([P, 9 * D + D + 1], F16)
            bias_t = constp.tile([P, 4], F32)
            ident = constp.tile([P, P], F16)
            nc.sync.dma_start(idx_t[:], idx_d[:])
            nc.sync.dma_start(wts_t[:], wts_d[:])
            nc.sync.dma_start(bias_t[:], bias_d[:])
            make_identity(nc, ident[:])
            smc = constp.tile([P, N_CACHE_BLKS * 2 * gw], F16)
            nc.sync.dma_start(smc[:, :N_CACHE_BLKS * gw],
                              smat_d[:, :N_CACHE_BLKS * gw])
            nc.sync.dma_start(
                smc[:, N_CACHE_BLKS * gw:],
                smat_d[:, N_BLOCKS * gw:(N_BLOCKS + N_CACHE_BLKS) * gw])

            def wslice(i):  # i-th [P, D] weight block (lhsT layout [fi, fo])
                return wts_t[:, i * D:(i + 1) * D]

            w2_ap = wts_t[:, 10 * D:10 * D + 1]

            # ---- big feature-major activations [P, 6400] f16 ------------
            tA = bigp.tile([P, BLK_NODES], F16, tag="tA")
            tB = bigp.tile([P, BLK_NODES], F16, tag="tB")
            tC = bigp.tile([P, BLK_NODES], F16, tag="tC")
            tD = bigp.tile([P, BLK_NODES], F16, tag="tD")
            nc.sync.dma_start(tA[:], x0fm[:])

            bncA = [dramp.tile([HSLAB, D], F16, tag=f"bncA{i}", name=f"bncA{i}")
                    for i in range(5)]
            bncB = [dramp.tile([HSLAB, D], F16, tag=f"bncB{i}", name=f"bncB{i}")
                    for i in range(5)]

            qctr = [0]

            def issue_call(c, h, src_tab):
                """One SWDGE gather (+ smat stream) covering BPC blocks."""
                nb = min(BPC, N_BLOCKS - c * BPC)
                cw = nb * gw
                g0 = (h * N_BLOCKS + c * BPC)
                g = gatp.tile([P, BPC * gw], F16, tag="g")
                nc.gpsimd.dma_gather(
                    out_ap=g[:, :cw].rearrange("p (n d) -> p n d", d=D),
                    in_ap=src_tab[:],
                    idxs_ap=idx_t[:, g0 * gcols:(g0 + nb) * gcols],
                    num_idxs=cw,
                    num_idxs_reg=cw,
                    elem_size=D,
                    queue_num=qctr[0] % N_QUEUES,
                    single_packet=False,
                )
                qctr[0] += 1
                if c * BPC < N_CACHE_BLKS:  # cache region is call-aligned
                    s = smc[:, (h * N_CACHE_BLKS + c * BPC) * gw:
                            (h * N_CACHE_BLKS + c * BPC + nb) * gw]
                else:
                    st = selp.tile([P, BPC * gw], F16, tag="s")
                    nc.sync.dma_start(
                        st[:, :cw], smat_d[:, g0 * gw:(g0 + nb) * gw])
                    s = st[:]
                return g, s

            def mm_chain(g, s, bb):
                """S-matmul accumulation for block bb within its call."""
                ps = psp.tile([P, P], F32, tag="ps", space="PSUM")
                for t in range(tp):
                    o = (bb * tp + t) * P
                    nc.tensor.matmul(
                        out=ps[:], lhsT=g[:, o:o + P], rhs=s[:, o:o + P],
                        start=(t == 0), stop=(t == tp - 1),
                    )
                return ps

            def table_block(j, src_fm, blo, bhi):
                """Transpose block j to node-major rows, store to bnc DRAM."""
                pt = pstp.tile([P, P], F16, tag="pt", space="PSUM")
                nc.tensor.transpose(pt[:], src_fm[:, j * P:(j + 1) * P],
                                    ident[:])
                nm = nmp.tile([P, P], F16, tag="nm")
                nc.vector.tensor_copy(out=nm[:], in_=pt[:])
                if j < N_HB:
                    nc.scalar.dma_start(blo[j * P:(j + 1) * P, :], nm[:])
                else:
                    jj = j - N_HB
                    nc.scalar.dma_start(bhi[jj * P:(jj + 1) * P, :], nm[:])

            def fire_ag(j, blo, bhi, tabA, tabB):
                if j == N_HB - 1:
                    nc.gpsimd.collective_compute(
                        "AllGather", mybir.AluOpType.bypass,
                        replica_groups=rg, ins=[blo[:]], outs=[tabA[:]])
                elif j == N_BLOCKS - 1:
                    nc.gpsimd.collective_compute(
                        "AllGather", mybir.AluOpType.bypass,
                        replica_groups=rg, ins=[bhi[:]], outs=[tabB[:]])

            def prop_phaseA(srcA, out_fm, tx0_fm=None):
                """out = psA (prop1) or 2*psA - tx0 (prop2), all blocks."""
                g = s = None
                for b in range(N_BLOCKS):
                    if b % BPC == 0:
                        g, s = issue_call(b // BPC, 0, srcA)
                    ps = mm_chain(g, s, b % BPC)
                    osl = out_fm[:, b * P:(b + 1) * P]
                    if tx0_fm is None:
                        nc.vector.tensor_copy(out=osl, in_=ps[:])
                    else:
                        nc.vector.scalar_tensor_tensor(
                            out=osl, in0=ps[:], scalar=2.0,
                            in1=tx0_fm[:, b * P:(b + 1) * P],
                            op0=mybir.AluOpType.mult,
                            op1=mybir.AluOpType.subtract)

            def prop1_phaseB(srcB, out_fm, blo, bhi, tabA, tabB):
                """out += psB per block; build + AllGather table(out).

                Transposes lag one block behind the evict so the PE never
                waits on the DVE eviction of the same block."""
                g = s = None
                for i in range(N_BLOCKS + 1):
                    if i < N_BLOCKS:
                        if i % BPC == 0:
                            g, s = issue_call(i // BPC, 1, srcB)
                        ps = mm_chain(g, s, i % BPC)
                        osl = out_fm[:, i * P:(i + 1) * P]
                        nc.vector.tensor_tensor(
                            out=osl, in0=osl, in1=ps[:],
                            op=mybir.AluOpType.add)
                    if i >= 1:
                        j = i - 1
                        table_block(j, out_fm, blo, bhi)
                        fire_ag(j, blo, bhi, tabA, tabB)

            def prop2_phaseB(srcB, tx2_fm, tx0_fm, tx1_fm, h_fm, wb, bias_col,
                             table=None, mlp=False):
                """tx2 += 2*psB per block, fused cheb_out (+ table | MLP).

                Pipeline: prop MMs for block i, cheb MMs for block i-1,
                transpose/MLP for block i-2."""
                relu = table is not None
                g = s = None
                for i in range(N_BLOCKS + 2):
                    if i < N_BLOCKS:
                        if i % BPC == 0:
                            g, s = issue_call(i // BPC, 1, srcB)
                        ps = mm_chain(g, s, i % BPC)
                        osl = tx2_fm[:, i * P:(i + 1) * P]
                        nc.vector.scalar_tensor_tensor(
                            out=osl, in0=ps[:], scalar=2.0, in1=osl,
                            op0=mybir.AluOpType.mult,
                            op1=mybir.AluOpType.add)
                    if 1 <= i <= N_BLOCKS:
                        j = i - 1
                        po = pcp.tile([P, P], F32, tag="po", space="PSUM")
                        for k, txk in enumerate((tx0_fm, tx1_fm, tx2_fm)):
                            nc.tensor.matmul(
                                out=po[:], lhsT=wslice(wb + k),
                                rhs=txk[:, j * P:(j + 1) * P],
                                start=(k == 0), stop=(k == 2))
                        hsl = h_fm[:, j * P:(j + 1) * P]
                        if relu:
                            nc.scalar.activation(
                                hsl, po[:],
                                mybir.ActivationFunctionType.Relu,
                                bias=bias_t[:, bias_col:bias_col + 1],
                                scale=1.0)
                        else:
                            # layer 3: b_out is folded into the MLP bias
                            nc.vector.tensor_copy(out=hsl, in_=po[:])
                    if i >= 2:
                        j = i - 2
                        if table is not None:
                            blo, bhi, tabA, tabB = table
                            table_block(j, h_fm, blo, bhi)
                            fire_ag(j, blo, bhi, tabA, tabB)
                        if mlp:
                            pm = pcp.tile([P, P], F32, tag="po", space="PSUM")
                            nc.tensor.matmul(
                                out=pm[:], lhsT=wslice(9),
                                rhs=h_fm[:, j * P:(j + 1) * P],
                                start=True, stop=True)
                            h4 = nmp.tile([P, P], F16, tag="h4")
                            nc.scalar.activation(
                                h4[:], pm[:],
                                mybir.ActivationFunctionType.Relu,
                                bias=bias_t[:, 3:4], scale=1.0)
                            p2 = p2p.tile([1, P], F32, tag="p2", space="PSUM")
                            nc.tensor.matmul(out=p2[:], lhsT=w2_ap, rhs=h4[:],
                                             start=True, stop=True)
                            yo = nmp.tile([1, P], F32, tag="yo")
                            nc.scalar.activation(
                                yo[:], p2[:],
                                mybir.ActivationFunctionType.Sigmoid,
                                bias=b2_val, scale=1.0)
                            nc.scalar.dma_start(
                                y_d[:, j * P:(j + 1) * P], yo[:1, :])

            # ================= layer 1 =================
            prop_phaseA(xA, tB)                       # Tx1 partial (half A)
            prop1_phaseB(xB, tB, bncA[0], bncB[0], tabsA[0], tabsB[0])
            prop_phaseA(tabsA[0], tC, tx0_fm=tA)      # Tx2 partial
            prop2_phaseB(tabsB[0], tC, tA, tB, tD, 0, 0,
                         table=(bncA[1], bncB[1], tabsA[1], tabsB[1]))

            # ================= layer 2 =================
            prop_phaseA(tabsA[1], tB)
            prop1_phaseB(tabsB[1], tB, bncA[2], bncB[2], tabsA[2], tabsB[2])
            prop_phaseA(tabsA[2], tC, tx0_fm=tD)
            prop2_phaseB(tabsB[2], tC, tD, tB, tA, 3, 1,
                         table=(bncA[3], bncB[3], tabsA[3], tabsB[3]))

            # ================= layer 3 =================
            prop_phaseA(tabsA[3], tB)
            prop1_phaseB(tabsB[3], tB, bncA[4], bncB[4], tabsA[4], tabsB[4])
            prop_phaseA(tabsA[4], tC, tx0_fm=tA)
            prop2_phaseB(tabsB[4], tC, tA, tB, tD, 6, 2, table=None, mlp=True)

    nc.finalize()
    return nc


_CACHE = {}


def kernel(x, edge_index, edge_weight, W_in, b_in, W_hid, b_hid, W_out, b_out,
           mlp_w1, mlp_b1, bn_gamma, bn_beta, bn_mean, bn_var, mlp_w2, mlp_b2,
           _trace=False):
    x = np.asarray(x, dtype=np.float32)
    t_half, idx_all, smat_all = _preprocess(
        np.asarray(edge_index), np.asarray(edge_weight))

    b2_val = float(np.asarray(mlp_b2, np.float32).reshape(-1)[0])
    cache_key = (t_half, b2_val)
    if cache_key in _CACHE:
        nc = _CACHE[cache_key]
    else:
        nc = _build_program(t_half, b2_val)
        _CACHE[cache_key] = nc

    # ---- host-side tensor prep ----------------------------------------
    xpad = np.zeros((N_PAD, D), dtype=np.float32)
    xpad[:N_NODES] = x
    x16_np = xpad.astype(npf16)

    # BN folding: y = s*(h@W1 + b1) + t -> W1' = W1*s, b1' = b1*s + t
    s = (np.asarray(bn_gamma, np.float32)
         / np.sqrt(np.asarray(bn_var, np.float32) + BN_EPS))
    t_ = np.asarray(bn_beta, np.float32) - np.asarray(bn_mean, np.float32) * s
    w1p = np.asarray(mlp_w1, np.float32) * s[None, :]
    b1p = np.asarray(mlp_b1, np.float32) * s + t_
    # fold the layer-3 ChebConv bias through the MLP first linear:
    # (h3 + b_out) @ W1' + b1' = h3 @ W1' + (b_out @ W1' + b1')
    b1p = np.asarray(b_out, np.float32) @ w1p + b1p

    wts = np.zeros((P, 9 * D + D + 1), dtype=npf16)
    for i, W in enumerate((W_in, W_hid, W_out)):
        W = np.asarray(W, np.float32)
        for k in range(K):
            wts[:, (i * K + k) * D:(i * K + k + 1) * D] = W[k].astype(npf16)
    wts[:, 9 * D:10 * D] = w1p.astype(npf16)
    wts[:, 10 * D:10 * D + 1] = np.asarray(mlp_w2, np.float32).astype(npf16)

    biases = np.zeros((P, 4), dtype=np.float32)
    biases[:, 0] = np.asarray(b_in, np.float32)
    biases[:, 1] = np.asarray(b_hid, np.float32)
    biases[:, 2] = 0.0
    biases[:, 3] = b1p

    in_maps = []
    for c in range(N_CORES):
        own = np.concatenate([
            x16_np[c * HSLAB:(c + 1) * HSLAB],
            x16_np[HALF + c * HSLAB:HALF + (c + 1) * HSLAB],
        ], axis=0)  # [6400, 128]
        in_maps.append({
            "xA": x16_np[:HALF],
            "xB": x16_np[HALF:],
            "x0fm": np.ascontiguousarray(own.T),
            "idx": idx_all[c],
            "smat": smat_all[c],
            "wts": wts,
            "bias": biases,
        })

    res = run_bass_kernel_spmd(nc, in_maps, list(range(N_CORES)), trace=_trace)
    y_full = np.zeros(N_PAD, dtype=np.float32)
    for c in range(N_CORES):
        yc = res.results[c]["y"][0]
        y_full[c * HSLAB:(c + 1) * HSLAB] = yc[:HSLAB]
        y_full[HALF + c * HSLAB:HALF + (c + 1) * HSLAB] = yc[HSLAB:]
    out = y_full[:N_NODES, None].astype(np.float32)
    if _trace:
        kernel._last_results = res
    return out


# revision 11
# speedup vs baseline: 1.3354x; 1.3354x over previous
"""DeepChebNet (3-layer ChebConv K=3 + MLP head) on 8 Trainium2 NeuronCores.

v3.1 strategy (1D node partition, fused per-block pipeline):
  - 50000 nodes padded to 51200, two 25600-row half-tables (A/B); each core
    owns 3200 nodes of each half (50 x 128-node blocks: 25 "lo" + 25 "hi").
  - Edges grouped by (src half, dst block), idx/smat laid out
    (half, block)-contiguous; one SWDGE gather call per (block, half)
    (~1152 rows) -- the measured sweet spot for the gather ucode
    (~2.5-3 ns/row; larger calls take a slow path).
  - Each propagate: per block, two gather calls (src half A and B) feed an
    18-matmul PSUM accumulation against precomputed selection matrices
    (smat); blocks 0..11 of both halves stay SBUF-resident (one-time load).
  - The Chebyshev recurrence evict (2*ps - Tx0), cheb_out (W0/W1/W2 matmuls
    + bias/ReLU), the PE transpose to node-major table rows, bnc stores,
    and the final MLP head + sigmoid are fused into the per-block loop
    (lag-1/lag-2 software pipeline) so the PE stays warm.
  - Half-table AllGathers fire mid-loop (after block 24 / 49); the lo-half
    collective overlaps the hi-half compute of the same propagate.
  - smat loads ride the sync HWDGE ring; table/y stores ride the scalar
    HWDGE ring (separate FIFOs, no head-of-line blocking).
"""
import numpy as np

import concourse.bacc as bacc
import concourse.bass as bass
import concourse.mybir as mybir
import concourse.tile as tile
from concourse.bass_utils import run_bass_kernel_spmd
from concourse.masks import make_identity

# problem constants (hardcoded per harness contract)
N_NODES = 50000
N_EDGES = 800000
D = 128
K = 3
BN_EPS = 1e-5

N_CORES = 8
P = 128
N_PAD = 51200
HALF = 25600            # rows per half-table (< 32768: int16-indexable)
HSLAB = 3200            # per-core nodes per half
BLK_NODES = 6400        # per-core nodes
N_BLOCKS = 50           # per-core 128-node blocks (25 lo + 25 hi)
N_HB = 25               # blocks per half per core
N_QUEUES = 4
N_CACHE_BLKS = 12       # smat slabs for blocks 0..11 (both halves) SBUF-resident

F16 = mybir.dt.float16
F32 = mybir.dt.float32
npf16 = np.float16


def _owner_block(n):
    """global node id -> (core, block 0..49) under the lo/hi layout."""
    lo = n < HALF
    core = np.where(lo, n // HSLAB, (n - HALF) // HSLAB)
    blk = np.where(lo, (n % HSLAB) // P, N_HB + ((n - HALF) % HSLAB) // P)
    return core, blk


def _preprocess(edge_index, edge_weight):
    """Graph partition + per-core edge streams (dst-block / src-half)."""
    src = np.asarray(edge_index[0], dtype=np.int64)
    dst = np.asarray(edge_index[1], dtype=np.int64)
    w = np.asarray(edge_weight, dtype=np.float32)

    deg = np.bincount(src, weights=w.astype(np.float64), minlength=N_NODES)
    deg = deg.astype(np.float32)
    degs = np.sqrt(np.maximum(deg, 1e-38))
    dinv = np.where(deg > 0, 1.0 / degs, 0.0).astype(np.float32)
    norm = (-dinv[src] * w * dinv[dst]).astype(np.float32)

    core, blk = _owner_block(dst)
    half = (src >= HALF).astype(np.int64)
    key = (core * N_BLOCKS + blk) * 2 + half
    order = np.argsort(key, kind="stable")
    src_s, dst_s, norm_s, key_s = src[order], dst[order], norm[order], key[order]

    n_groups = N_CORES * N_BLOCKS * 2    # 800 (core, block, half) groups
    bounds = np.searchsorted(key_s, np.arange(n_groups + 1))
    counts = bounds[1:] - bounds[:-1]
    t_half = max(1, int(np.max((counts + P - 1) // P)))  # tiles per group

    idx_all, smat_all = [], []
    n_tiles = N_BLOCKS * 2 * t_half
    for c in range(N_CORES):
        gslots = t_half * P
        n_slots = N_BLOCKS * 2 * gslots
        e_src = np.zeros(n_slots, dtype=np.int16)
        e_dstl = np.zeros(n_slots, dtype=np.int64)
        e_norm = np.zeros(n_slots, dtype=np.float32)
        e_live = np.zeros(n_slots, dtype=bool)
        for b in range(N_BLOCKS):
            for h in range(2):
                gidx = (c * N_BLOCKS + b) * 2 + h
                lo, hi = bounds[gidx], bounds[gidx + 1]
                n = hi - lo
                base = (b * 2 + h) * gslots
                e_src[base:base + n] = (src_s[lo:hi] - h * HALF).astype(np.int16)
                e_dstl[base:base + n] = dst_s[lo:hi] % P
                e_norm[base:base + n] = norm_s[lo:hi]
                e_live[base:base + n] = True
                # pad slots gather row 0 (real packets; their S column is 0)
        # int16 idx stream: per (h, b) group, flat i -> (row i%16, col i//16),
        # replicated across the 8 groups of 16 partitions.
        n_grp = N_BLOCKS * 2
        per_grp = np.transpose(
            e_src.reshape(N_BLOCKS, 2, gslots // 16, 16), (1, 0, 3, 2)
        ).reshape(n_grp, 16, gslots // 16)
        arr = np.concatenate([per_grp[i] for i in range(n_grp)], axis=1)
        idx16 = np.zeros((P, n_grp * (gslots // 16)), dtype=np.int16)
        for gs in range(8):
            idx16[gs * 16:(gs + 1) * 16, :] = arr
        idx_all.append(np.ascontiguousarray(idx16))
        # precomputed selection matrices: smat[p, gt*P + j] =
        #   norm_e if (tile gt, lane p) holds edge e with dst_local j
        slot = np.nonzero(e_live)[0]
        gt, lane = slot // P, slot % P
        s_all = np.zeros(n_tiles * P * P, dtype=np.float16)
        s_all[(gt * P + lane) * P + e_dstl[slot]] = e_norm[slot]
        # reorder tiles (b, h, t) -> (h, b, t) to match the idx layout
        smat = np.ascontiguousarray(
            s_all.reshape(N_BLOCKS, 2, t_half, P, P).transpose(3, 1, 0, 2, 4)
            .reshape(P, -1))
        smat_all.append(smat)
    return t_half, idx_all, smat_all


def _build_program(t_half, b2_val):
    """Build the SPMD Bass program (identical across cores)."""
    nc = bacc.Bacc("TRN2", target_bir_lowering=False, debug=False,
                   num_devices=N_CORES, num_swdge_queues=N_QUEUES)

    tp = t_half                  # tiles per (block, half) group
    gw = tp * P                  # gather / S width per group
    gcols = gw // 16             # idx columns per group
    n_tiles = N_BLOCKS * 2 * tp

    # ---- I/O -----------------------------------------------------------
    xA = nc.dram_tensor("xA", [HALF, D], F16, kind="ExternalInput")
    xB = nc.dram_tensor("xB", [HALF, D], F16, kind="ExternalInput")
    x0fm = nc.dram_tensor("x0fm", [P, BLK_NODES], F16, kind="ExternalInput")
    idx_d = nc.dram_tensor("idx", [P, 2 * N_BLOCKS * gcols], mybir.dt.int16,
                           kind="ExternalInput")
    smat_d = nc.dram_tensor("smat", [P, n_tiles * P], F16,
                            kind="ExternalInput")
    wts_d = nc.dram_tensor("wts", [P, 9 * D + D + 1], F16, kind="ExternalInput")
    bias_d = nc.dram_tensor("bias", [P, 4], F32, kind="ExternalInput")
    y_d = nc.dram_tensor("y", [1, BLK_NODES], F32, kind="ExternalOutput")

    tabsA = [nc.dram_tensor(f"tabA{i}", [HALF, D], F16, addr_space="Shared")
             for i in range(5)]
    tabsB = [nc.dram_tensor(f"tabB{i}", [HALF, D], F16, addr_space="Shared")
             for i in range(5)]
    rg = [list(range(N_CORES))]

    with tile.TileContext(nc) as tc:
        with (
            tc.tile_pool(name="const", bufs=1) as constp,
            tc.tile_pool(name="big", bufs=1) as bigp,
            tc.tile_pool(name="gat", bufs=10) as gatp,
            tc.tile_pool(name="sel", bufs=6) as selp,
            tc.tile_pool(name="nm", bufs=4) as nmp,
            tc.tile_pool(name="ps", bufs=4, space="PSUM") as psp,
            tc.tile_pool(name="pc", bufs=2, space="PSUM") as pcp,
            tc.tile_pool(name="pt", bufs=1, space="PSUM") as pstp,
            tc.tile_pool(name="p2", bufs=1, space="PSUM") as p2p,
            tc.tile_pool(name="dram", bufs=1, space="DRAM") as dramp,
        ):
            # ---- load constants -----------------------------------------
            idx_t = constp.tile([P, 2 * N_BLOCKS * gcols], mybir.dt.int16)
            wts_t = constp.tile([P, 9 * D + D + 1], F16)
            bias_t = constp.tile([P, 4], F32)
            ident = constp.tile([P, P], F16)
            nc.sync.dma_start(idx_t[:], idx_d[:])
            nc.sync.dma_start(wts_t[:], wts_d[:])
            nc.sync.dma_start(bias_t[:], bias_d[:])
            make_identity(nc, ident[:])
            smc = constp.tile([P, N_CACHE_BLKS * 2 * gw], F16)
            nc.sync.dma_start(smc[:, :N_CACHE_BLKS * gw],
                              smat_d[:, :N_CACHE_BLKS * gw])
            nc.sync.dma_start(
                smc[:, N_CACHE_BLKS * gw:],
                smat_d[:, N_BLOCKS * gw:(N_BLOCKS + N_CACHE_BLKS) * gw])

            def wslice(i):  # i-th [P, D] weight block (lhsT layout [fi, fo])
                return wts_t[:, i * D:(i + 1) * D]

            w2_ap = wts_t[:, 10 * D:10 * D + 1]

            # ---- big feature-major activations [P, 6400] f16 ------------
            tA = bigp.tile([P, BLK_NODES], F16, tag="tA")
            tB = bigp.tile([P, BLK_NODES], F16, tag="tB")
            tC = bigp.tile([P, BLK_NODES], F16, tag="tC")
            tD = bigp.tile([P, BLK_NODES], F16, tag="tD")
            nc.sync.dma_start(tA[:], x0fm[:])

            bncA = [dramp.tile([HSLAB, D], F16, tag=f"bncA{i}", name=f"bncA{i}")
                    for i in range(5)]
            bncB = [dramp.tile([HSLAB, D], F16, tag=f"bncB{i}", name=f"bncB{i}")
                    for i in range(5)]

            qctr = [0]

            def gather_group(b, h, src_tab):
                """SWDGE gather of one (block, half) edge group."""
                g = gatp.tile([P, gw], F16, tag="g")
                g0 = h * N_BLOCKS + b
                nc.gpsimd.dma_gather(
                    out_ap=g[:].rearrange("p (n d) -> p n d", d=D),
                    in_ap=src_tab[:],
                    idxs_ap=idx_t[:, g0 * gcols:(g0 + 1) * gcols],
                    num_idxs=gw,
                    num_idxs_reg=gw,
                    elem_size=D,
                    queue_num=qctr[0] % N_QUEUES,
                    single_packet=False,
                )
                qctr[0] += 1
                return g

            def smat_group(b, h):
                g0 = h * N_BLOCKS + b
                if b < N_CACHE_BLKS:
                    return smc[:, (h * N_CACHE_BLKS + b) * gw:
                               (h * N_CACHE_BLKS + b + 1) * gw]
                st = selp.tile([P, gw], F16, tag="s")
                nc.sync.dma_start(st[:], smat_d[:, g0 * gw:(g0 + 1) * gw])
                return st[:]

            def accum_block(b, srcA, srcB):
                """Both halves' gathers + 2*tp-matmul PSUM accumulation."""
                gA = gather_group(b, 0, srcA)
                gB = gather_group(b, 1, srcB)
                sA = smat_group(b, 0)
                sB = smat_group(b, 1)
                ps = psp.tile([P, P], F32, tag="ps", space="PSUM")
                for t in range(tp):
                    nc.tensor.matmul(
                        out=ps[:], lhsT=gA[:, t * P:(t + 1) * P],
                        rhs=sA[:, t * P:(t + 1) * P],
                        start=(t == 0), stop=False)
                for t in range(tp):
                    nc.tensor.matmul(
                        out=ps[:], lhsT=gB[:, t * P:(t + 1) * P],
                        rhs=sB[:, t * P:(t + 1) * P],
                        start=False, stop=(t == tp - 1))
                return ps

            def table_block(j, src_fm, blo, bhi):
                """Transpose block j to node-major rows, store to bnc DRAM."""
                pt = pstp.tile([P, P], F16, tag="pt", space="PSUM")
                nc.tensor.transpose(pt[:], src_fm[:, j * P:(j + 1) * P],
                                    ident[:])
                nm = nmp.tile([P, P], F16, tag="nm")
                nc.vector.tensor_copy(out=nm[:], in_=pt[:])
                if j < N_HB:
                    nc.scalar.dma_start(blo[j * P:(j + 1) * P, :], nm[:])
                else:
                    jj = j - N_HB
                    nc.scalar.dma_start(bhi[jj * P:(jj + 1) * P, :], nm[:])

            def fire_ag(j, blo, bhi, tabA, tabB):
                if j == N_HB - 1:
                    nc.gpsimd.collective_compute(
                        "AllGather", mybir.AluOpType.bypass,
                        replica_groups=rg, ins=[blo[:]], outs=[tabA[:]])
                elif j == N_BLOCKS - 1:
                    nc.gpsimd.collective_compute(
                        "AllGather", mybir.AluOpType.bypass,
                        replica_groups=rg, ins=[bhi[:]], outs=[tabB[:]])

            def prop1(srcA, srcB, out_fm, blo, bhi, tabA, tabB):
                """Tx1 = A_hat @ src; build + AllGather table(Tx1).

                Table transposes lag one block behind the evict."""
                for i in range(N_BLOCKS + 1):
                    if i < N_BLOCKS:
                        ps = accum_block(i, srcA, srcB)
                        nc.vector.tensor_copy(
                            out=out_fm[:, i * P:(i + 1) * P], in_=ps[:])
                    if i >= 1:
                        j = i - 1
                        table_block(j, out_fm, blo, bhi)
                        fire_ag(j, blo, bhi, tabA, tabB)

            def prop2(srcA, srcB, tx2_fm, tx0_fm, tx1_fm, h_fm, wb, bias_col,
                      table=None, mlp=False):
                """Tx2 = 2*A_hat@Tx1 - Tx0, fused cheb_out (+ table | MLP).

                Pipeline: prop MMs for block i, cheb MMs for block i-1,
                transpose/MLP for block i-2."""
                relu = table is not None
                for i in range(N_BLOCKS + 2):
                    if i < N_BLOCKS:
                        ps = accum_block(i, srcA, srcB)
                        nc.vector.scalar_tensor_tensor(
                            out=tx2_fm[:, i * P:(i + 1) * P], in0=ps[:],
                            scalar=2.0, in1=tx0_fm[:, i * P:(i + 1) * P],
                            op0=mybir.AluOpType.mult,
                            op1=mybir.AluOpType.subtract)
                    if 1 <= i <= N_BLOCKS:
                        j = i - 1
                        po = pcp.tile([P, P], F32, tag="po", space="PSUM")
                        for k, txk in enumerate((tx0_fm, tx1_fm, tx2_fm)):
                            nc.tensor.matmul(
                                out=po[:], lhsT=wslice(wb + k),
                                rhs=txk[:, j * P:(j + 1) * P],
                                start=(k == 0), stop=(k == 2))
                        hsl = h_fm[:, j * P:(j + 1) * P]
                        if relu:
                            nc.scalar.activation(
                                hsl, po[:],
                                mybir.ActivationFunctionType.Relu,
                                bias=bias_t[:, bias_col:bias_col + 1],
                                scale=1.0)
                        else:
                            # layer 3: b_out is folded into the MLP bias
                            nc.vector.tensor_copy(out=hsl, in_=po[:])
                    if i >= 2:
                        j = i - 2
                        if table is not None:
                            blo, bhi, tabA, tabB = table
                            table_block(j, h_fm, blo, bhi)
                            fire_ag(j, blo, bhi, tabA, tabB)
                        if mlp:
                            pm = pcp.tile([P, P], F32, tag="po", space="PSUM")
                            nc.tensor.matmul(
                                out=pm[:], lhsT=wslice(9),
                                rhs=h_fm[:, j * P:(j + 1) * P],
                                start=True, stop=True)
                            h4 = nmp.tile([P, P], F16, tag="h4")
                            nc.scalar.activation(
                                h4[:], pm[:],
                                mybir.ActivationFunctionType.Relu,
                                bias=bias_t[:, 3:4], scale=1.0)
                            p2 = p2p.tile([1, P], F32, tag="p2", space="PSUM")
                            nc.tensor.matmul(out=p2[:], lhsT=w2_ap, rhs=h4[:],
                                             start=True, stop=True)
                            yo = nmp.tile([1, P], F32, tag="yo")
                            nc.scalar.activation(
                                yo[:], p2[:],
                                mybir.ActivationFunctionType.Sigmoid,
                                bias=b2_val, scale=1.0)
                            nc.scalar.dma_start(
                                y_d[:, j * P:(j + 1) * P], yo[:1, :])

            # ================= layer 1 =================
            prop1(xA, xB, tB, bncA[0], bncB[0], tabsA[0], tabsB[0])
            prop2(tabsA[0], tabsB[0], tC, tA, tB, tD, 0, 0,
                  table=(bncA[1], bncB[1], tabsA[1], tabsB[1]))

            # ================= layer 2 =================
            prop1(tabsA[1], tabsB[1], tB, bncA[2], bncB[2],
                  tabsA[2], tabsB[2])
            prop2(tabsA[2], tabsB[2], tC, tD, tB, tA, 3, 1,
                  table=(bncA[3], bncB[3], tabsA[3], tabsB[3]))

            # ================= layer 3 =================
            prop1(tabsA[3], tabsB[3], tB, bncA[4], bncB[4],
                  tabsA[4], tabsB[4])
            prop2(tabsA[4], tabsB[4], tC, tA, tB, tD, 6, 2,
                  table=None, mlp=True)

    nc.finalize()
    return nc


_CACHE = {}


def kernel(x, edge_index, edge_weight, W_in, b_in, W_hid, b_hid, W_out, b_out,
           mlp_w1, mlp_b1, bn_gamma, bn_beta, bn_mean, bn_var, mlp_w2, mlp_b2,
           _trace=False):
    x = np.asarray(x, dtype=np.float32)
    t_half, idx_all, smat_all = _preprocess(
        np.asarray(edge_index), np.asarray(edge_weight))

    b2_val = float(np.asarray(mlp_b2, np.float32).reshape(-1)[0])
    cache_key = (t_half, b2_val)
    if cache_key in _CACHE:
        nc = _CACHE[cache_key]
    else:
        nc = _build_program(t_half, b2_val)
        _CACHE[cache_key] = nc

    # ---- host-side tensor prep ----------------------------------------
    xpad = np.zeros((N_PAD, D), dtype=np.float32)
    xpad[:N_NODES] = x
    x16_np = xpad.astype(npf16)

    # BN folding: y = s*(h@W1 + b1) + t -> W1' = W1*s, b1' = b1*s + t
    s = (np.asarray(bn_gamma, np.float32)
         / np.sqrt(np.asarray(bn_var, np.float32) + BN_EPS))
    t_ = np.asarray(bn_beta, np.float32) - np.asarray(bn_mean, np.float32) * s
    w1p = np.asarray(mlp_w1, np.float32) * s[None, :]
    b1p = np.asarray(mlp_b1, np.float32) * s + t_
    # fold the layer-3 ChebConv bias through the MLP first linear:
    # (h3 + b_out) @ W1' + b1' = h3 @ W1' + (b_out @ W1' + b1')
    b1p = np.asarray(b_out, np.float32) @ w1p + b1p

    wts = np.zeros((P, 9 * D + D + 1), dtype=npf16)
    for i, W in enumerate((W_in, W_hid, W_out)):
        W = np.asarray(W, np.float32)
        for k in range(K):
            wts[:, (i * K + k) * D:(i * K + k + 1) * D] = W[k].astype(npf16)
    wts[:, 9 * D:10 * D] = w1p.astype(npf16)
    wts[:, 10 * D:10 * D + 1] = np.asarray(mlp_w2, np.float32).astype(npf16)

    biases = np.zeros((P, 4), dtype=np.float32)
    biases[:, 0] = np.asarray(b_in, np.float32)
    biases[:, 1] = np.asarray(b_hid, np.float32)
    biases[:, 2] = 0.0
    biases[:, 3] = b1p

    in_maps = []
    for c in range(N_CORES):
        own = np.concatenate([
            x16_np[c * HSLAB:(c + 1) * HSLAB],
            x16_np[HALF + c * HSLAB:HALF + (c + 1) * HSLAB],
        ], axis=0)  # [6400, 128]
        in_maps.append({
            "xA": x16_np[:HALF],
            "xB": x16_np[HALF:],
            "x0fm": np.ascontiguousarray(own.T),
            "idx": idx_all[c],
            "smat": smat_all[c],
            "wts": wts,
            "bias": biases,
        })

    res = run_bass_kernel_spmd(nc, in_maps, list(range(N_CORES)), trace=_trace)
    y_full = np.zeros(N_PAD, dtype=np.float32)
    for c in range(N_CORES):
        yc = res.results[c]["y"][0]
        y_full[c * HSLAB:(c + 1) * HSLAB] = yc[:HSLAB]
        y_full[HALF + c * HSLAB:HALF + (c + 1) * HSLAB] = yc[HSLAB:]
    out = y_full[:N_NODES, None].astype(np.float32)
    if _trace:
        kernel._last_results = res
    return out
